# revision 7
# baseline (speedup 1.0000x reference)
"""Trainium2 Bass kernel for nn_BilinearAttention (B=128,P=49,E=2048,D=512,A=512).

Strategy (8 NeuronCores, one chip):
  * Wb (the 512^3 bilinear weight) is tensor-parallel sharded over its
    OUTPUT dim k: core c owns k in [64c, 64c+64).
  * att1 = enc@We^T is data-parallel over batch (16 batches/core), then
    AllGathered (f16) so every core has att1 for all 128 batches.
  * u = att2 x Wb_shard and bil = att1 x u are computed per-core for all
    batches over the local k-shard.  att_partial[b,p] = sum_k relu(bil+bb)*Wf
    is a per-core partial sum over k -> AllReduce([128,49]) gives full att.
  * softmax(att) -> alpha is computed redundantly on every core; the final
    attention-weighted encoding is data-parallel over batch again (each core
    contracts its own encoder slice with its own alphas, selected via a
    one-hot "sel" input so the program is identical on every core).

All big matmuls run in f16 (fp32 PSUM accumulation); bias adds, softmax and
outputs are fp32.  Host-side preprocessing (numpy, untimed) does the
transposes and f16 casts.
"""

import numpy as np
import ml_dtypes

import concourse.bass as bass
import concourse.mybir as mybir
import concourse.tile as tile
from concourse import bacc
from concourse.bass_utils import run_bass_kernel_spmd
from concourse.masks import make_identity

B, P, E, D, A = 128, 49, 2048, 512, 512
NCORES = 8
BS = B // NCORES          # 16 batches per core
KSH = A // NCORES         # 64 bilinear output dims per core
BP = BS * P               # 784 (b,p) rows per core
ET = E // 128             # 16 e-tiles
IT = A // 128             # 4 i/j tiles
NH = 2                    # att1 N split halves (784 -> 2x392)
NHW = BP // NH            # 392
MCH = KSH * A // 128      # 256 u chunks of 128 (k,i) rows
QP = B // 2               # 64 batch pairs
QPB = 10                  # batch-pairs per PSUM bank in att partial
NQB = (QP + QPB - 1) // QPB  # 7 banks
AMT = 7                   # awe k-tiles
AMK = BP // AMT           # 112

f32 = mybir.dt.float32
f16 = mybir.dt.float16
AF = mybir.ActivationFunctionType
RG = [list(range(NCORES))]

_CACHE = {}


def _build_nc():
    nc = bacc.Bacc("TRN2", target_bir_lowering=False, debug=False,
                   num_devices=NCORES)
    d = {}

    def inp(name, shape, dt=f16):
        d[name] = nc.dram_tensor(name, shape, dt, kind="ExternalInput").ap()

    inp("encT", [E, BP])          # own enc slice, [e, (b,p)]
    inp("encN", [BP, E])          # own enc slice, [(b,p), e]
    inp("WeT", [E, A])            # We transposed -> [e, i]
    inp("decT", [D, B])           # decoder_hidden transposed [d, b] (all b)
    inp("WdT", [D, A])            # Wd transposed [d, j]
    inp("WbT", [A, KSH * A])      # own k-shard of Wb: [j, (k,i)]
    inp("be", [A, 1], f32)
    inp("bd", [A, 1], f32)
    inp("bb2", [128, 1], f32)     # own bb shard duplicated [bb_sh; bb_sh]
    inp("Wf2", [128, 2])          # block-diag Wf shard (f16)
    inp("sel", [B, BS])           # one-hot selector of own batches (f16)
    out_awe = nc.dram_tensor("out_awe", [BS, E], f32, kind="ExternalOutput").ap()
    out_alpha = nc.dram_tensor("out_alpha", [B, P], f32, kind="ExternalOutput").ap()

    with tile.TileContext(nc) as tc:
        _kernel(tc, d, out_awe, out_alpha)
    nc.compile()
    return nc


def _kernel(tc, d, out_awe, out_alpha):
    nc = tc.nc
    with (
        tc.tile_pool(name="cpool", bufs=1) as cpool,
        tc.tile_pool(name="spool", bufs=4) as spool,
        tc.tile_pool(name="wbpool", bufs=2) as wbpool,
        tc.tile_pool(name="encpool", bufs=4) as encpool,
        tc.tile_pool(name="dpool", bufs=1, space="DRAM") as dpool,
    ):
        # ---------------- constant loads ----------------
        WeT_sb = cpool.tile([128, ET * A], f16)
        for et in range(ET):
            nc.sync.dma_start(out=WeT_sb[:, et * A:(et + 1) * A],
                              in_=d["WeT"][et * 128:(et + 1) * 128, :])
        encT_sb = cpool.tile([128, ET * BP], f16)
        for et in range(ET):
            nc.sync.dma_start(out=encT_sb[:, et * BP:(et + 1) * BP],
                              in_=d["encT"][et * 128:(et + 1) * 128, :])
        decT_sb = cpool.tile([128, IT * B], f16)
        WdT_sb = cpool.tile([128, IT * A], f16)
        for dt_ in range(IT):
            nc.sync.dma_start(out=decT_sb[:, dt_ * B:(dt_ + 1) * B],
                              in_=d["decT"][dt_ * 128:(dt_ + 1) * 128, :])
            nc.sync.dma_start(out=WdT_sb[:, dt_ * A:(dt_ + 1) * A],
                              in_=d["WdT"][dt_ * 128:(dt_ + 1) * 128, :])
        be_sb = cpool.tile([128, IT], f32)
        bd_sb = cpool.tile([128, IT], f32)
        for ic in range(IT):
            nc.sync.dma_start(out=be_sb[:, ic:ic + 1],
                              in_=d["be"][ic * 128:(ic + 1) * 128, :])
            nc.sync.dma_start(out=bd_sb[:, ic:ic + 1],
                              in_=d["bd"][ic * 128:(ic + 1) * 128, :])
        bb2_sb = cpool.tile([128, 1], f32)
        nc.sync.dma_start(out=bb2_sb[:], in_=d["bb2"][:])
        Wf2_sb = cpool.tile([128, 2], f16)
        nc.sync.dma_start(out=Wf2_sb[:], in_=d["Wf2"][:])
        sel_sb = cpool.tile([B, BS], f16)
        nc.sync.dma_start(out=sel_sb[:], in_=d["sel"][:])

        att2T_sb = cpool.tile([128, IT * B], f16)       # [j, (jt, b)]
        att1T_sb = cpool.tile([128, IT * NCORES * BP], f16)  # [i, (it, r, bp)]
        u_sb = cpool.tile([128, MCH * B], f16)          # [(k,i)%128, (m, b)]

        # ---------------- att2 = Wd @ dec + bd (f16, all batches) ----------
        with tc.tile_pool(name="pa", bufs=2, space="PSUM") as pa:
            for jc in range(IT):
                ps_a2 = pa.tile([128, B], f32, tag="a2")
                for dt_ in range(IT):
                    nc.tensor.matmul(
                        ps_a2[:],
                        WdT_sb[:, dt_ * A + jc * 128: dt_ * A + (jc + 1) * 128],
                        decT_sb[:, dt_ * B:(dt_ + 1) * B],
                        start=(dt_ == 0), stop=(dt_ == IT - 1))
                nc.scalar.activation(att2T_sb[:, jc * B:(jc + 1) * B], ps_a2[:],
                                     AF.Identity, bias=bd_sb[:, jc:jc + 1])

            # ------------ att1 own slice:  [i, (b,p)] + be ------------------
            a1b = dpool.tile([IT, 128, BP], f16)        # AllGather contribution
            for ic in range(IT):
                for nh in range(NH):
                    n0 = nh * NHW
                    ps_a1 = pa.tile([128, NHW], f32, tag="a1")
                    for et in range(ET):
                        nc.tensor.matmul(
                            ps_a1[:],
                            WeT_sb[:, et * A + ic * 128: et * A + (ic + 1) * 128],
                            encT_sb[:, et * BP + n0: et * BP + n0 + NHW],
                            start=(et == 0), stop=(et == ET - 1))
                    a1s = spool.tile([128, NHW], f16, tag="a1s")
                    nc.scalar.activation(a1s[:], ps_a1[:], AF.Identity,
                                         bias=be_sb[:, ic:ic + 1])
                    nc.sync.dma_start(out=a1b[ic, :, n0:n0 + NHW], in_=a1s[:])

        # ---------------- AllGather att1 (f16, 6.4MB out) ------------------
        a1g = dpool.tile([NCORES, IT, 128, BP], f16, addr_space="Shared")
        nc.gpsimd.collective_compute(
            "AllGather", mybir.AluOpType.bypass, replica_groups=RG,
            ins=[a1b.opt()], outs=[a1g.opt()])
        for r in range(NCORES):
            for it in range(IT):
                nc.sync.dma_start(
                    out=att1T_sb[:, (it * NCORES + r) * BP:(it * NCORES + r + 1) * BP],
                    in_=a1g[r, it, :, :])

        # ---------------- u = att2 x Wb_shard  ([(k,i), b], f16) -----------
        # (runs on PE while the AllGather is in flight on the TOPSP/SDMA side)
        with tc.tile_pool(name="pu", bufs=4, space="PSUM") as pu:
            for blk in range(MCH // 8):
                wbt = []
                for jt in range(IT):
                    w = wbpool.tile([128, 1024], f16, tag=f"wb{jt}")
                    nc.sync.dma_start(
                        out=w[:],
                        in_=d["WbT"][jt * 128:(jt + 1) * 128,
                                     blk * 1024:(blk + 1) * 1024])
                    wbt.append(w)
                for m8 in range(8):
                    m = blk * 8 + m8
                    ps_u = pu.tile([128, B], f32, tag="u")
                    for jt in range(IT):
                        nc.tensor.matmul(
                            ps_u[:],
                            wbt[jt][:, m8 * 128:(m8 + 1) * 128],
                            att2T_sb[:, jt * B:(jt + 1) * B],
                            start=(jt == 0), stop=(jt == IT - 1))
                    nc.vector.tensor_copy(u_sb[:, m * B:(m + 1) * B], ps_u[:])

        # ---------------- bil + relu + Wf partial reduction -----------------
        # u_sb free index = m*128 + b with m = 4k + it  ->  k*512 + it*128 + b
        u_r = u_sb[:].rearrange("p (k y) -> p k y", y=4 * B)
        attb = dpool.tile([B, P], f32)
        attb_r = attb[:].rearrange("(q h) p -> h q p", h=2)
        with (
            tc.tile_pool(name="pb", bufs=4, space="PSUM") as pbp,
            tc.tile_pool(name="patt", bufs=2, space="PSUM") as pattp,
        ):
            att_ps = None
            for q in range(QP):
                if q % QPB == 0:
                    att_ps = pattp.tile([2, QPB * P], f32, tag="attps")
                ps_b = pbp.tile([128, P], f32, tag="bil")
                for half in range(2):
                    b = 2 * q + half
                    r, bl = b // BS, b % BS
                    for it in range(IT):
                        nc.tensor.matmul(
                            ps_b[half * KSH:(half + 1) * KSH, :],
                            u_r[:, :, it * B + b],
                            att1T_sb[:, (it * NCORES + r) * BP + bl * P:
                                     (it * NCORES + r) * BP + (bl + 1) * P],
                            start=(it == 0), stop=(it == IT - 1))
                rl = spool.tile([128, P], f16, tag="rl")
                nc.scalar.activation(rl[:], ps_b[:], AF.Relu, bias=bb2_sb[:])
                nc.tensor.matmul(att_ps[:, (q % QPB) * P:(q % QPB + 1) * P],
                                 Wf2_sb[:], rl[:], start=True, stop=True)
                if q % QPB == QPB - 1 or q == QP - 1:
                    j = q // QPB
                    n = (q % QPB) + 1
                    att_st = spool.tile([2, QPB * P], f32, tag="attst")
                    nc.vector.tensor_copy(att_st[:, :n * P], att_ps[:, :n * P])
                    nc.sync.dma_start(out=attb_r[:, j * QPB: j * QPB + n, :],
                                      in_=att_st[:, :n * P])

        # ---------------- AllReduce att + softmax ---------------------------
        attr = dpool.tile([B, P], f32, addr_space="Shared")
        nc.gpsimd.collective_compute(
            "AllReduce", mybir.AluOpType.add, replica_groups=RG,
            ins=[attb.opt()], outs=[attr.opt()])
        att_sb = cpool.tile([B, P], f32)
        nc.sync.dma_start(out=att_sb[:], in_=attr[:])
        mxn = cpool.tile([B, 1], f32)
        nc.vector.tensor_reduce(mxn[:], att_sb[:], axis=mybir.AxisListType.X,
                                op=mybir.AluOpType.max, negate=True)
        exp_sb = cpool.tile([B, P], f32)
        se = cpool.tile([B, 1], f32)
        nc.scalar.activation(exp_sb[:], att_sb[:], AF.Exp, bias=mxn[:],
                             accum_out=se[:])
        rs = cpool.tile([B, 1], f32)
        nc.vector.reciprocal(rs[:], se[:])
        alpha_sb = cpool.tile([B, P], f32)
        nc.vector.tensor_scalar_mul(alpha_sb[:], exp_sb[:], rs[:])
        nc.sync.dma_start(out=out_alpha, in_=alpha_sb[:])
        alpha_bf = cpool.tile([B, P], f16)
        nc.vector.tensor_copy(alpha_bf[:], alpha_sb[:])

        # ---------------- awe = alpha-weighted encoder sum ------------------
        ident = cpool.tile([128, 128], f16)
        make_identity(nc, ident[:])
        with (
            tc.tile_pool(name="pown", bufs=1, space="PSUM") as pown,
            tc.tile_pool(name="ptr", bufs=2, space="PSUM") as ptr,
            tc.tile_pool(name="pawe", bufs=2, space="PSUM") as pawe,
        ):
            own_ps = pown.tile([BS, P], f32)
            nc.tensor.matmul(own_ps[:], sel_sb[:], alpha_bf[:],
                             start=True, stop=True)
            alpha_own_sb = cpool.tile([BS, P], f16)
            nc.vector.tensor_copy(alpha_own_sb[:], own_ps[:])
            AmT = cpool.tile([BS, BP], f16)
            nc.vector.memset(AmT[:], 0.0)
            for j in range(BS):
                nc.sync.dma_start(out=AmT[j:j + 1, j * P:(j + 1) * P],
                                  in_=alpha_own_sb[j:j + 1, :])
            Am_sb = cpool.tile([AMK, AMT * BS], f16)
            for t in range(AMT):
                tp = ptr.tile([AMK, BS], f16, tag="tr")
                nc.tensor.transpose(tp[:], AmT[:, t * AMK:(t + 1) * AMK],
                                    ident[0:BS, 0:BS])
                nc.vector.tensor_copy(Am_sb[:, t * BS:(t + 1) * BS], tp[:])
            for nk in range(4):
                ps_awe = pawe.tile([BS, 512], f32, tag="awe")
                for t in range(AMT):
                    ench = encpool.tile([AMK, 512], f16, tag="ench")
                    nc.sync.dma_start(
                        out=ench[:],
                        in_=d["encN"][t * AMK:(t + 1) * AMK,
                                      nk * 512:(nk + 1) * 512])
                    nc.tensor.matmul(ps_awe[:], Am_sb[:, t * BS:(t + 1) * BS],
                                     ench[:], start=(t == 0), stop=(t == AMT - 1))
                awe_st = spool.tile([BS, 512], f32, tag="awest")
                nc.vector.tensor_copy(awe_st[:], ps_awe[:])
                nc.sync.dma_start(out=out_awe[:, nk * 512:(nk + 1) * 512],
                                  in_=awe_st[:])


def _bf(x):
    return np.ascontiguousarray(np.asarray(x, np.float32).astype(np.float16))


def make_in_maps(encoder_out, decoder_hidden, We, be, Wd, bd, Wb, bb, Wf, bf):
    encoder_out = np.asarray(encoder_out, np.float32)
    decoder_hidden = np.asarray(decoder_hidden, np.float32)
    We = np.asarray(We, np.float32)
    Wd = np.asarray(Wd, np.float32)
    Wb = np.asarray(Wb, np.float32)
    be = np.asarray(be, np.float32)
    bd = np.asarray(bd, np.float32)
    bb = np.asarray(bb, np.float32)
    Wf = np.asarray(Wf, np.float32)

    WeT = _bf(We.T)                       # [E, A]
    decT = _bf(decoder_hidden.T)          # [D, B]
    WdT = _bf(Wd.T)                       # [D, A]
    be_c = np.ascontiguousarray(be.reshape(A, 1))
    bd_c = np.ascontiguousarray(bd.reshape(A, 1))

    in_maps = []
    for c in range(NCORES):
        enc_c = encoder_out[c * BS:(c + 1) * BS].reshape(BP, E)
        k0 = c * KSH
        Wb_c = Wb[k0:k0 + KSH].reshape(KSH * A, A)     # [(k,i), j]
        bb_sh = bb[k0:k0 + KSH]
        Wf_sh = Wf[0, k0:k0 + KSH]
        Wf2 = np.zeros((128, 2), np.float32)
        Wf2[:KSH, 0] = Wf_sh
        Wf2[KSH:, 1] = Wf_sh
        sel = np.zeros((B, BS), np.float32)
        sel[np.arange(c * BS, (c + 1) * BS), np.arange(BS)] = 1.0
        in_maps.append({
            "encT": _bf(enc_c.T),
            "encN": _bf(enc_c),
            "WeT": WeT,
            "decT": decT,
            "WdT": WdT,
            "WbT": _bf(Wb_c.T),
            "be": be_c,
            "bd": bd_c,
            "bb2": np.ascontiguousarray(
                np.concatenate([bb_sh, bb_sh]).reshape(128, 1)),
            "Wf2": _bf(Wf2),
            "sel": _bf(sel),
        })
    return in_maps


def get_nc():
    if "nc" not in _CACHE:
        _CACHE["nc"] = _build_nc()
    return _CACHE["nc"]


def assemble(results):
    awe = np.concatenate([np.asarray(results[c]["out_awe"], np.float32)
                          for c in range(NCORES)], axis=0)
    alpha = np.asarray(results[0]["out_alpha"], np.float32)
    return awe, alpha


def kernel(encoder_out, decoder_hidden, We, be, Wd, bd, Wb, bb, Wf, bf,
           **run_kwargs):
    nc = get_nc()
    in_maps = make_in_maps(encoder_out, decoder_hidden, We, be, Wd, bd,
                           Wb, bb, Wf, bf)
    res = run_bass_kernel_spmd(nc, in_maps, core_ids=list(range(NCORES)),
                               **run_kwargs)
    _CACHE["last_result"] = res
    return assemble(res.results)
